# revision 9
# baseline (speedup 1.0000x reference)
"""DSA varlen sparse attention for Trainium2, 8 NeuronCores — v4.

Token-sharded (256 tokens/core), K/V replicated, dense-S + sparse-mask
formulation (softmax Z cancels against the renormalization):
   out[t,h] = (sum_j exp(s[j,t]) * tsd[j,t] * V[j,h]) / (sum_j exp*tsd)

v4 structural changes vs the previous kernel:
  - Host pre-sorts each token's (topk_idx, topk_score) pairs by index
    (pure permutation; the output is invariant to per-token slot order).
    Duplicate-index merging then becomes a segmented suffix-sum scan
    over adjacent slots (O(K log K) on DVE, ~3us) instead of the O(K^2)
    all-pairs is_equal matrix (~22us).  Non-first slots of each run are
    parked out of range so local_scatter sees unique indices.
  - ACT runs ONLY the 32 exp instructions (normalize moved to GPSIMD,
    tsdT drains to DVE); ACT is the critical engine at ~33us busy.
  - V is loaded per-head (h-major DRAM layout) interleaved with K heads
    so AV(h) unblocks at ~10-23us instead of ~30us; AV matmuls and tsd
    transposes fill PE slack in the exp-paced S stream (which also keeps
    the PE p-state ramp warm).
  - Per-(h,t) output DMAs so the tail after the last exp is short.
"""

import numpy as np
import ml_dtypes
from contextlib import ExitStack

T, H, D, DV, TK = 2048, 8, 128, 128, 64
NCORES = 8
TC = T // NCORES          # 256 tokens per core
P = 128
TCH = TC // P             # 2 token chunks of 128
JC = T // P               # 16 key chunks of 128
SCALE = float(D) ** -0.5
HALF = 1024               # local_scatter num_elems limit is < 2048
G = 4                     # score jc-chunks per PSUM tile
NG = JC // G
NSM = 2 * TCH * TK + P

_CACHE = {}


def _build_program():
    import concourse.mybir as mybir
    import concourse.tile as tile
    from concourse import bacc

    dt = mybir.dt
    Alu = mybir.AluOpType
    Act = mybir.ActivationFunctionType

    nc = bacc.Bacc(None, target_bir_lowering=False, debug=False)
    names = {}
    with ExitStack() as ctx:
        tc = ctx.enter_context(tile.TileContext(nc))
        dram = ctx.enter_context(tc.tile_pool(name="dram", bufs=1, space="DRAM"))
        sb = ctx.enter_context(tc.tile_pool(name="sb", bufs=1))
        pT_pool = ctx.enter_context(tc.tile_pool(name="pTp", bufs=8))
        sm2 = ctx.enter_context(tc.tile_pool(name="sm2", bufs=2))
        sps = ctx.enter_context(tc.tile_pool(name="spsum", bufs=2, space="PSUM"))
        ops = ctx.enter_context(tc.tile_pool(name="opsum", bufs=4, space="PSUM"))

        # ---------------- DRAM I/O (bf16 data prepped host-side) ----------
        q_d = dram.tile([P, H * TC], dt.bfloat16, kind="ExternalInput")
        k_d = dram.tile([P, H * T], dt.bfloat16, kind="ExternalInput")
        # v is h-major so per-head loads are contiguous: [P, H, JC, 1+DV]
        v_d = dram.tile([P, H * JC * (1 + DV)], dt.bfloat16, kind="ExternalInput")
        sm_d = dram.tile([P, NSM], dt.int16, kind="ExternalInput")
        out_d = dram.tile([P, TCH, H * DV], dt.bfloat16, kind="ExternalOutput")
        names.update(
            q=q_d.name, k=k_d.name, v=v_d.name, sm=sm_d.name, out=out_d.name,
        )

        # ---------------- SBUF persistent ----------------
        kT = sb.tile([P, H, T], dt.bfloat16, tag="kT")
        vE = sb.tile([P, H, JC, 1 + DV], dt.bfloat16, tag="vE")
        qT = sb.tile([P, H, TC], dt.bfloat16, tag="qT")
        tsd = sb.tile([P, TCH, 2 * (HALF + 2)], dt.bfloat16, tag="tsd")
        tsdT = sb.tile([P, JC, TC], dt.bfloat16, tag="tsdT")
        smalls = sb.tile([P, NSM], dt.int16, tag="smalls")
        idx16 = smalls[:, 0 : TCH * TK].rearrange("p (a b) -> p a b", a=TCH)
        tsbf = (
            smalls[:, TCH * TK : 2 * TCH * TK]
            .bitcast(dt.bfloat16).rearrange("p (a b) -> p a b", a=TCH)
        )
        ident = smalls[:, 2 * TCH * TK :].bitcast(dt.bfloat16)
        outs = sb.tile([P, TCH, H * DV], dt.bfloat16, tag="outs")

        # ---------------- loads (single sync HWDGE queue; FIFO = priority)
        def kload(h, a, b):
            nc.sync.dma_start(
                out=kT[:, h, a:b], in_=k_d[:, h * T + a : h * T + b]
            )

        HVB = JC * (1 + DV)

        def vload(h):
            nc.sync.dma_start(
                out=vE[:, h].rearrange("p a b -> p (a b)"),
                in_=v_d[:, h * HVB : (h + 1) * HVB],
            )

        kload(0, 0, 512)
        nc.sync.dma_start(out=qT[:, 0, :], in_=q_d[:, 0:TC])
        nc.sync.dma_start(out=smalls[:], in_=sm_d[:])
        kload(0, 512, T)
        nc.sync.dma_start(
            out=qT[:, 1:H, :].rearrange("p a b -> p (a b)"), in_=q_d[:, TC:]
        )
        kload(1, 0, T)
        vload(0)
        vload(1)
        kload(2, 0, T)
        vload(2)
        kload(3, 0, T)
        vload(3)
        kload(4, 0, T)
        vload(4)
        kload(5, 0, T)
        vload(5)
        vload(6)
        kload(6, 0, T)
        vload(7)
        kload(7, 0, T)

        # ---------------- dedup: segmented suffix-sum over sorted slots ---
        # Host sorted each token's slots by index, so duplicate groups are
        # contiguous runs.  acc[k] accumulates the within-run suffix sum via
        # log2(TK) doubling steps; the first slot of each run ends up with
        # the full run sum.  Non-first slots (nf=1) are parked out of range
        # so the scatters see unique indices.
        acc = sm2.tile([P, TCH, TK], dt.bfloat16, tag="acc")
        nc.vector.tensor_copy(out=acc[:], in_=tsbf[:])
        tmp = sm2.tile([P, TCH, TK], dt.bfloat16, tag="tmp")
        same = sm2.tile([P, TCH, TK], dt.bfloat16, tag="same")
        with nc.allow_low_precision("duplicate-group sums have few terms"):
            s = 1
            while s < TK:
                w = TK - s
                for t in range(TCH):
                    nc.vector.tensor_tensor(
                        out=same[:, t, 0:w], in0=idx16[:, t, s:TK],
                        in1=idx16[:, t, 0:w], op=Alu.is_equal,
                    )
                    nc.vector.tensor_tensor(
                        out=tmp[:, t, 0:w], in0=same[:, t, 0:w],
                        in1=acc[:, t, s:TK], op=Alu.mult,
                    )
                    nc.vector.tensor_tensor(
                        out=acc[:, t, 0:w], in0=acc[:, t, 0:w],
                        in1=tmp[:, t, 0:w], op=Alu.add,
                    )
                s *= 2
        # nf[k] = 1 if slot k continues a run (not the first occurrence)
        nf = sm2.tile([P, TCH, TK], dt.float32, tag="nf")
        nc.vector.tensor_scalar(
            out=nf[:, :, 0:1], in0=idx16[:, :, 0:1],
            scalar1=0.0, scalar2=None, op0=Alu.mult,
        )
        nc.vector.tensor_tensor(
            out=nf[:, :, 1:TK], in0=idx16[:, :, 1:TK],
            in1=idx16[:, :, 0 : TK - 1], op=Alu.is_equal,
        )
        # bm = (idx+1) + nf*8192  (parks duplicate slots out of range)
        nfbig = sm2.tile([P, TCH, TK], dt.float32, tag="nfbig")
        nc.vector.tensor_scalar(
            out=nfbig[:], in0=nf[:], scalar1=8192.0, scalar2=None, op0=Alu.mult,
        )
        bm = sm2.tile([P, TCH, TK], dt.float32, tag="bm")
        nc.vector.tensor_scalar_add(out=bm[:], in0=idx16[:], scalar1=1.0)
        nc.vector.tensor_tensor(out=bm[:], in0=bm[:], in1=nfbig[:], op=Alu.add)
        # ilo = min(bm, HALF+1) - 1             in [0 .. HALF]
        # ihi = min(max(bm-HALF, 0), HALF+1)-1  in [-1 .. HALF]
        ilo = sm2.tile([P, TCH, TK], dt.int16, tag="ilo")
        ihi = sm2.tile([P, TCH, TK], dt.int16, tag="ihi")
        b2 = sm2.tile([P, TCH, TK], dt.float32, tag="b2")
        nc.vector.tensor_scalar(
            out=ilo[:], in0=bm[:], scalar1=float(HALF + 1), scalar2=-1.0,
            op0=Alu.min, op1=Alu.add,
        )
        nc.vector.tensor_scalar(
            out=b2[:], in0=bm[:], scalar1=float(-HALF), scalar2=0.0,
            op0=Alu.add, op1=Alu.max,
        )
        nc.vector.tensor_scalar(
            out=ihi[:], in0=b2[:], scalar1=float(HALF + 1), scalar2=-1.0,
            op0=Alu.min, op1=Alu.add,
        )

        for t in range(TCH):
            nc.gpsimd.local_scatter(
                out_ap=tsd[:, t, 0 : HALF + 2], data_ap=acc[:, t],
                idxs_ap=ilo[:, t], channels=P, num_elems=HALF + 2, num_idxs=TK,
            )
            nc.gpsimd.local_scatter(
                out_ap=tsd[:, t, HALF + 2 : 2 * HALF + 4], data_ap=acc[:, t],
                idxs_ap=ihi[:, t], channels=P, num_elems=HALF + 2, num_idxs=TK,
            )

        # ------------------ per-head S^T / exp / mask / AV ----------------
        pTs = []
        tr_psums = {}
        extras = []        # thunks emitting PE work into exp-paced slack slots

        def emit_transpose(t, jc):
            # pairs of transposes share a [P, 2, P] psum tile -> one drain
            if jc % 2 == 0:
                ps = ops.tile([P, 2, P], dt.bfloat16, tag="op")
                tr_psums[(t, jc // 2)] = ps
            else:
                ps = tr_psums[(t, jc // 2)]
            off = jc * P if jc < JC // 2 else HALF + 2 + (jc - JC // 2) * P
            nc.tensor.transpose(
                out=ps[:, jc % 2, :], in_=tsd[:, t, off : off + P],
                identity=ident[:],
            )

        def emit_drains(t):
            for pr in range(JC // 2):
                ps = tr_psums.pop((t, pr))
                nc.vector.tensor_copy(
                    out=tsdT[:, 2 * pr : 2 * pr + 2, t * P : (t + 1) * P],
                    in_=ps[:],
                )

        def emit_st_head(h, n_extra=0, plan=None):
            pT = pT_pool.tile([P, JC, TC], dt.bfloat16, tag="pT")
            pTs.append(pT)
            jc0 = 0
            for gsz in (plan or [G] * NG):
                sp = sps.tile([P, G, TC], dt.float32, tag="sp")
                for j in range(gsz):
                    nc.tensor.matmul(
                        out=sp[:, j, :],
                        lhsT=kT[:, h, (jc0 + j) * P : (jc0 + j + 1) * P],
                        rhs=qT[:, h, :],
                        start=True, stop=True,
                    )
                nc.scalar.activation(
                    out=pT[:, jc0 : jc0 + gsz, :], in_=sp[:, 0:gsz],
                    func=Act.Exp, scale=SCALE,
                )
                jc0 += gsz
                for _ in range(n_extra):
                    if extras:
                        extras.pop(0)()

        def emit_mask(h, g, t):
            pT = pTs[h]
            if t is None:
                nc.vector.tensor_tensor(
                    out=pT[:, g * G : (g + 1) * G, :],
                    in0=pT[:, g * G : (g + 1) * G, :],
                    in1=tsdT[:, g * G : (g + 1) * G, :],
                    op=Alu.mult,
                )
                return
            nc.vector.tensor_tensor(
                out=pT[:, g * G : (g + 1) * G, t * P : (t + 1) * P],
                in0=pT[:, g * G : (g + 1) * G, t * P : (t + 1) * P],
                in1=tsdT[:, g * G : (g + 1) * G, t * P : (t + 1) * P],
                op=Alu.mult,
            )

        def emit_masks(h, ts_=None):
            if ts_ is None:
                for g in range(NG):
                    emit_mask(h, g, None)
                return
            for t in ts_:
                for g in range(NG):
                    emit_mask(h, g, t)

        def emit_av(h, t):
            pT = pTs[h]
            op = ops.tile([P, 1 + DV], dt.float32, tag="op")
            for jc in range(JC):
                nc.tensor.matmul(
                    out=op[:],
                    lhsT=pT[:, jc, t * P : (t + 1) * P],
                    rhs=vE[:, h, jc, :],
                    start=(jc == 0), stop=(jc == JC - 1),
                )
            rec = sm2.tile([P, 1], dt.float32, tag="rec")
            nc.vector.reciprocal(out=rec[:], in_=op[:, 0:1])
            dst = outs[:, t, h * DV : (h + 1) * DV]
            nc.vector.tensor_scalar(
                out=dst, in0=op[:, 1 : 1 + DV],
                scalar1=rec[:], scalar2=None, op0=Alu.mult,
            )
            # last two heads ship as one combined DMA (per-piece DGE latency
            # would put ~3 serialized round-trips after the final normalize)
            if h < 6:
                nc.sync.dma_start(
                    out=out_d[:, t, h * DV : (h + 1) * DV], in_=dst
                )
            elif h == 7 and t == 1:
                nc.sync.dma_start(
                    out=out_d[:, :, 6 * DV :], in_=outs[:, :, 6 * DV :]
                )

        # transposes ride PE slack slots in heads 1-2 (after the scatters
        # land); AVs ride slots from head 3 on.
        emit_st_head(0)
        for jc in range(JC):
            extras.append(lambda jc=jc: emit_transpose(0, jc))
        emit_st_head(1, n_extra=4)
        for jc in range(JC):
            extras.append(lambda jc=jc: emit_transpose(1, jc))
        emit_st_head(2, n_extra=4)
        emit_drains(0)
        emit_masks(0, ts_=[0])
        emit_masks(1, ts_=[0])
        emit_drains(1)
        emit_masks(2, ts_=[0])
        emit_masks(0, ts_=[1])
        emit_masks(1, ts_=[1])
        emit_masks(2, ts_=[1])
        for h, t in [(0, 0), (0, 1), (1, 0), (1, 1), (2, 0), (2, 1)]:
            extras.append(lambda h=h, t=t: emit_av(h, t))
        for h in range(3, H):
            emit_st_head(h, n_extra=1)
            emit_masks(h)
            extras.append(lambda h=h: emit_av(h, 0))
            extras.append(lambda h=h: emit_av(h, 1))
        while extras:
            extras.pop(0)()

    nc.compile()
    return nc, names


def _get_program():
    if "prog" not in _CACHE:
        _CACHE["prog"] = _build_program()
    return _CACHE["prog"]


def _host_inputs(q, k, v, idx, ts):
    """Per-core in_maps (host-side shard/layout/dtype prep).

    Sorts each token's (index, score) slot pairs by index — a pure
    permutation (the reference output is invariant to slot order) that
    lets the device merge duplicates with an adjacent-slot scan.
    """
    bf16 = ml_dtypes.bfloat16
    identity = np.eye(P, dtype=np.float32).astype(bf16).view(np.int16)

    # kT[d, h, j] = K[j, h, d]
    k_full = np.ascontiguousarray(
        k.transpose(2, 1, 0).reshape(P, H * T)
    ).astype(bf16)
    # vE[p, h, jc, 0] = 1, vE[p, h, jc, 1:] = V[jc*128+p, h, :]
    v_r = v.reshape(JC, P, H, DV).transpose(1, 2, 0, 3)  # [P, H, JC, DV]
    v_full = np.ones((P, H, JC, 1 + DV), dtype=np.float32)
    v_full[:, :, :, 1:] = v_r
    v_full = v_full.reshape(P, H * JC * (1 + DV)).astype(bf16)

    idx = np.asarray(idx)
    order = np.argsort(idx, axis=1, kind="stable")
    idx_s = np.take_along_axis(idx, order, axis=1)
    ts_s = np.take_along_axis(np.asarray(ts), order, axis=1)

    maps = []
    for c in range(NCORES):
        sl = slice(c * TC, (c + 1) * TC)
        qc = q[sl].transpose(2, 1, 0).reshape(P, H * TC)
        ic = idx_s[sl].astype(np.int16).reshape(TCH, P, TK).transpose(1, 0, 2)
        tc_ = ts_s[sl].reshape(TCH, P, TK).transpose(1, 0, 2).astype(bf16)
        packed = np.concatenate(
            [
                ic.reshape(P, TCH * TK),
                tc_.reshape(P, TCH * TK).view(np.int16),
                identity,
            ],
            axis=1,
        )
        maps.append(
            dict(
                q=np.ascontiguousarray(qc).astype(bf16),
                k=k_full,
                v=v_full,
                sm=np.ascontiguousarray(packed),
            )
        )
    return maps


def kernel(q_packed, k_packed, v_packed, topk_indices, topk_scores):
    from concourse.bass_utils import run_bass_kernel_spmd

    q = np.asarray(q_packed, dtype=np.float32)
    k = np.asarray(k_packed, dtype=np.float32)
    v = np.asarray(v_packed, dtype=np.float32)
    idx = np.asarray(topk_indices)
    ts = np.asarray(topk_scores, dtype=np.float32)

    nc, names = _get_program()
    logical_maps = _host_inputs(q, k, v, idx, ts)
    in_maps = [{names[key]: arr for key, arr in m.items()} for m in logical_maps]

    res = run_bass_kernel_spmd(nc, in_maps, core_ids=list(range(NCORES)))
    outn = names["out"]
    parts = []
    for c in range(NCORES):
        oc = np.asarray(res.results[c][outn], dtype=np.float32)
        parts.append(oc.transpose(1, 0, 2).reshape(TC, H, DV))
    return np.concatenate(parts, axis=0).astype(np.float32)


if __name__ == "__main__":
    rng = np.random.default_rng(0)
    q = rng.standard_normal((T, H, D), dtype=np.float32)
    k = rng.standard_normal((T, H, D), dtype=np.float32)
    v = rng.standard_normal((T, H, DV), dtype=np.float32)
    idx = rng.integers(0, T, size=(T, TK), dtype=np.int64)
    ts = rng.random((T, TK), dtype=np.float32)
    out = kernel(q, k, v, idx, ts)
    print(out.shape, out.dtype)
